# revision 19
# baseline (speedup 1.0000x reference)
"""Trainium2 Bass kernel for nn_DAWN_35356170781342 (retrieval_knn layer).

kernel(**inputs) takes FULL unsharded inputs (as reference.setup_inputs()
produces) and returns the FULL output [1, 1024, 512] f32.

8-core sequence-parallel design:
  - core j owns tokens [128j, 128j+128)
  - neuron pools re-indexed per cell maps into "bc space" (64 cells x 16
    block = 1024 rows/circuit) on host; replicated per core (transposed-f32
    for score matmuls, bf16 for weighted sums)
  - per-token candidate window = 3x3 cell neighborhood; compaction/expansion
    via GPSIMD local_scatter with per-token index rows dma_gathered from
    small geometry tables
  - exact top-k thresholds via iterative vector.max + match_replace
  - K^T and V (bf16) AllGathered; attention computed on transposed scores
    sT[k, q] so attn needs no transpose; additive mask; exp without max-sub;
    softmax denominator via ones-matmul
"""

import numpy as np
import ml_dtypes

import concourse.bass as bass
import concourse.mybir as mybir
from concourse.bass_types import AP as BassAP
from concourse import bacc
from concourse.tile import TileContext
from concourse.bass_utils import run_bass_kernel_spmd
from concourse.masks import make_identity

F32 = mybir.dt.float32
BF16 = mybir.dt.bfloat16
I16 = mybir.dt.int16
I32 = mybir.dt.int32
AF = mybir.ActivationFunctionType
OP = mybir.AluOpType

B, S, D, NH = 1, 1024, 512, 8
DH = D // NH
CELLS, BLOCK = 8, 16
NBC = CELLS * CELLS * BLOCK          # 1024 bc rows per circuit
NCOMP = 144
MAXK_QK, MAXK_V, MAXK_KNOW = 32, 32, 64
POS_MIN, POS_RANGE = -3.0, 6.0
NCORES = 8
TT = 128

_CACHE = {}
TRACE = False
LAST_EXEC_NS = None
LAST_RESULT = None


def _geometry_tables():
    T1 = np.full((36, NBC), -1, np.int16)
    T2 = np.full((36, 256), 0, np.int16)
    for a in range(1, 7):
        for b in range(1, 7):
            g = (a - 1) * 6 + (b - 1)
            for r in range(3):
                for c in range(3):
                    cell = (a - 1 + r) * 8 + (b - 1 + c)
                    for j in range(16):
                        dense = cell * 16 + j
                        comp = (r * 3 + c) * 16 + j
                        T1[g, dense] = comp
                        T2[g, comp] = dense
    return T1, T2


class KB:
    """Kernel build context."""

    def __init__(self, nc, pools):
        self.nc = nc
        for kk, v in pools.items():
            setattr(self, kk, v)

    def ln(self, x_t, name):
        nc, pool = self.nc, self.work
        ssum = pool.tile([TT, 1], F32, tag=f"{name}_s", name=f"{name}_s")
        nc.vector.reduce_sum(out=ssum, in_=x_t, axis=mybir.AxisListType.X)
        negmean = pool.tile([TT, 1], F32, tag=f"{name}_nm", name=f"{name}_nm")
        nc.vector.tensor_scalar_mul(negmean, ssum, -1.0 / D)
        xc = pool.tile([TT, D], F32, tag="ln_xc", name=f"{name}_xc")
        nc.vector.tensor_scalar(out=xc, in0=x_t, scalar1=negmean, scalar2=None,
                                op0=OP.add)
        sq = pool.tile([TT, D], F32, tag="ln_sq", name=f"{name}_sq")
        sumsq = pool.tile([TT, 1], F32, tag=f"{name}_ss", name=f"{name}_ss")
        nc.scalar.activation(out=sq, in_=xc, func=AF.Square, accum_out=sumsq)
        rstd = pool.tile([TT, 1], F32, tag=f"{name}_rs", name=f"{name}_rs")
        nc.vector.tensor_scalar(out=rstd, in0=sumsq, scalar1=1.0 / D,
                                scalar2=1e-6, op0=OP.mult, op1=OP.add)
        nc.scalar.activation(out=rstd, in_=rstd, func=AF.Sqrt)
        nc.vector.reciprocal(out=rstd, in_=rstd)
        xn = pool.tile([TT, D], F32, tag="ln_xn", name=f"{name}_xn")
        nc.vector.tensor_scalar(out=xn, in0=xc, scalar1=rstd, scalar2=None,
                                op0=OP.mult)
        return xn

    def pe_transpose(self, src, nchunks, name, out_dt=F32, pool=None,
                     also_f32=False):
        nc = self.nc
        pool = pool or self.big
        dst = pool.tile([TT, nchunks * TT], out_dt, tag=f"{name}_T",
                        name=f"{name}_T")
        dst2 = None
        if also_f32:
            dst2 = pool.tile([TT, nchunks * TT], F32, tag=f"{name}_Tf",
                             name=f"{name}_Tf")
        for c in range(nchunks):
            ps = self.ps128.tile([TT, TT], src.dtype, tag="mm128",
                                 name=f"{name}_ps{c}")
            idt = self.ident if src.dtype == F32 else self.ident_b
            nc.tensor.transpose(ps, src[:, c * TT:(c + 1) * TT], idt)
            nc.scalar.activation(out=dst[:, c * TT:(c + 1) * TT], in_=ps,
                                 func=AF.Copy)
            if also_f32:
                nc.vector.tensor_copy(out=dst2[:, c * TT:(c + 1) * TT], in_=ps)
        return (dst, dst2) if also_f32 else dst

    def wrap_idx(self, g_scr_row, name):
        nc = self.nc
        idxw = self.work.tile([128, 8], I16, tag=f"{name}_iw", name=f"{name}_iw")
        src = g_scr_row.rearrange("(s p) -> p s", p=16)
        for kk in range(8):
            eng = nc.scalar if kk % 2 == 0 else nc.sync
            eng.dma_start(out=idxw[16 * kk:16 * kk + 16, :], in_=src)
        return idxw

    def gather_tables(self, idxw, name):
        nc = self.nc
        idx1 = self.idxp.tile([128, NBC], I16, tag="i1", name=f"{name}_i1")
        nc.gpsimd.dma_gather(
            out_ap=idx1.rearrange("p (a b) -> p a b", a=1), in_ap=self.t1_ext,
            idxs_ap=idxw, num_idxs=128, num_idxs_reg=128, elem_size=NBC)
        idx2 = self.idxp.tile([128, 256], I16, tag=f"{name}_i2",
                              name=f"{name}_i2")
        nc.gpsimd.dma_gather(
            out_ap=idx2.rearrange("p (a b) -> p a b", a=1), in_ap=self.t2_ext,
            idxs_ap=idxw, num_idxs=128, num_idxs_reg=128, elem_size=256)
        return idx1, idx2

    def scores(self, xnT, bct_tiles, name):
        nc = self.nc
        s = self.scorep.tile([TT, NBC], BF16, tag="scores", name=f"{name}_s")
        for half in range(2):
            ps = self.ps512.tile([TT, 512], F32, tag="mm512",
                                 name=f"{name}_ps{half}")
            for c in range(4):
                nc.tensor.matmul(ps, xnT[:, c * TT:(c + 1) * TT],
                                 bct_tiles[c][:, half * 512:(half + 1) * 512],
                                 start=(c == 0), stop=(c == 3))
            nc.scalar.activation(out=s[:, half * 512:(half + 1) * 512],
                                 in_=ps, func=AF.Copy)
        return s

    def compact(self, s_t, idx1, name):
        nc = self.nc
        compb = self.work.tile([TT, NCOMP], BF16, tag=f"{name}_cb",
                               name=f"{name}_cb")
        nc.gpsimd.local_scatter(
            out_ap=compb, data_ap=s_t, idxs_ap=idx1,
            channels=128, num_elems=NCOMP, num_idxs=NBC)
        comp = self.work.tile([TT, NCOMP], F32, tag=f"{name}_c", name=f"{name}_c")
        nc.vector.tensor_copy(out=comp, in_=compb)
        return comp

    def topk(self, comp, kk, name):
        nc, pool = self.nc, self.work
        work = pool.tile([TT, NCOMP], F32, tag=f"{name}_w", name=f"{name}_w")
        nc.vector.tensor_copy(out=work, in_=comp)
        rounds = kk // 8
        mx = pool.tile([TT, 8 * rounds], F32, tag=f"{name}_m", name=f"{name}_m")
        for r in range(rounds):
            nc.vector.max(out=mx[:, 8 * r:8 * r + 8], in_=work)
            if r < rounds - 1:
                nc.vector.match_replace(out=work,
                                        in_to_replace=mx[:, 8 * r:8 * r + 8],
                                        in_values=work, imm_value=-1e30)
        return mx[:, 0:1], mx[:, 8 * rounds - 1:8 * rounds]

    def gate(self, comp, tau, m0, theta, name):
        nc, pool = self.nc, self.work
        negtau = pool.tile([TT, 1], F32, tag=f"{name}_nt", name=f"{name}_nt")
        nc.vector.tensor_scalar_mul(negtau, tau, -1.0)
        raw = pool.tile([TT, NCOMP], F32, tag=f"{name}_raw", name=f"{name}_raw")
        nc.vector.tensor_scalar(out=raw, in0=comp, scalar1=negtau, scalar2=None,
                                op0=OP.add)
        rmin = pool.tile([TT, NCOMP], F32, tag=f"{name}_rm", name=f"{name}_rm")
        nc.vector.tensor_scalar_min(rmin, raw, 0.0)
        e1 = pool.tile([TT, NCOMP], F32, tag=f"{name}_e1", name=f"{name}_e1")
        nc.scalar.activation(out=e1, in_=rmin, func=AF.Exp)
        nc.vector.tensor_scalar_mul(e1, e1, 1e-8)
        gate = pool.tile([TT, NCOMP], F32, tag=f"{name}_g", name=f"{name}_g")
        nc.vector.tensor_tensor(out=gate, in0=raw, in1=e1, op=OP.max)
        eg1 = pool.tile([TT, NCOMP], F32, tag=f"{name}_eg", name=f"{name}_eg")
        nc.scalar.activation(out=eg1, in_=gate, func=AF.Exp)   # = eg + 1
        keep = pool.tile([TT, NCOMP], F32, tag=f"{name}_k", name=f"{name}_k")
        nc.vector.tensor_scalar(out=keep, in0=comp, scalar1=theta, scalar2=None,
                                op0=OP.is_ge)
        w_u = pool.tile([TT, NCOMP], F32, tag=f"{name}_wu", name=f"{name}_wu")
        nc.vector.tensor_scalar_add(w_u, eg1, -1.0)
        nc.vector.tensor_tensor(out=w_u, in0=w_u, in1=keep, op=OP.mult)
        ssum = pool.tile([TT, 1], F32, tag=f"{name}_ss", name=f"{name}_ss")
        nc.vector.reduce_sum(out=ssum, in_=w_u, axis=mybir.AxisListType.X)
        sc = pool.tile([TT, 4], F32, tag=f"{name}_sc", name=f"{name}_sc")
        rawm = sc[:, 0:1]; rminm = sc[:, 1:2]; egm = sc[:, 2:3]; tnh = sc[:, 3:4]
        nc.vector.tensor_tensor(out=rawm, in0=m0, in1=negtau, op=OP.add)
        nc.vector.tensor_scalar_min(rminm, rawm, 0.0)
        nc.scalar.activation(out=rminm, in_=rminm, func=AF.Exp)
        nc.vector.tensor_scalar_mul(rminm, rminm, 1e-8)
        nc.vector.tensor_tensor(out=egm, in0=rawm, in1=rminm, op=OP.max)
        nc.scalar.activation(out=egm, in_=egm, func=AF.Exp)
        nc.vector.tensor_scalar_add(egm, egm, -1.0)
        nc.scalar.activation(out=tnh, in_=egm, func=AF.Tanh)
        den = pool.tile([TT, 1], F32, tag=f"{name}_dn", name=f"{name}_dn")
        nc.vector.tensor_scalar_add(den, ssum, 1e-8)
        nc.vector.reciprocal(out=den, in_=den)
        scale = pool.tile([TT, 1], F32, tag=f"{name}_scl", name=f"{name}_scl")
        nc.vector.tensor_tensor(out=scale, in0=tnh, in1=den, op=OP.mult)
        wq = pool.tile([TT, NCOMP], F32, tag=f"{name}_wq", name=f"{name}_wq")
        nc.vector.tensor_tensor(out=wq, in0=w_u, in1=comp, op=OP.mult)
        wb = pool.tile([TT, NCOMP], BF16, tag=f"{name}_wb", name=f"{name}_wb")
        nc.vector.tensor_scalar(out=wb, in0=wq, scalar1=scale, scalar2=None,
                                op0=OP.mult)
        return wb

    def expand_transpose(self, wc_b, idx2, name):
        nc = self.nc
        wd = self.idxp.tile([TT, NBC], BF16, tag="wd", name=f"{name}_wd")
        nc.gpsimd.local_scatter(out_ap=wd, data_ap=wc_b,
                                idxs_ap=idx2[:, 0:NCOMP], channels=128,
                                num_elems=NBC, num_idxs=NCOMP)
        return self.pe_transpose(wd, 8, f"{name}w", out_dt=BF16)

    def cellgrid(self, pos2, name):
        nc, pool = self.nc, self.work
        q = pool.tile([TT, 2], F32, tag=f"{name}_q", name=f"{name}_q")
        nc.scalar.activation(out=q, in_=pos2, func=AF.Copy, bias=3.5,
                             scale=float(CELLS / POS_RANGE))
        ci = pool.tile([TT, 2], I32, tag=f"{name}_ci", name=f"{name}_ci")
        nc.vector.tensor_copy(out=ci, in_=q)
        nc.vector.tensor_scalar_max(ci, ci, 1)
        nc.vector.tensor_scalar_min(ci, ci, 6)
        g = pool.tile([TT, 1], I32, tag=f"{name}_gi", name=f"{name}_gi")
        cx6 = pool.tile([TT, 1], I32, tag=f"{name}_c6", name=f"{name}_c6")
        nc.vector.tensor_scalar(out=cx6, in0=ci[:, 0:1], scalar1=6, scalar2=-7,
                                op0=OP.mult, op1=OP.add)
        nc.vector.tensor_tensor(out=g, in0=cx6, in1=ci[:, 1:2], op=OP.add)
        g16 = pool.tile([TT, 1], I16, tag=f"{name}_g16", name=f"{name}_g16")
        nc.vector.tensor_copy(out=g16, in_=g)
        return g16


def build_kernel(n_cores=NCORES, debug=False):
    nc = bacc.Bacc("TRN2", target_bir_lowering=False, debug=False,
                   num_devices=n_cores)

    def dp(name, shape, dt, out=False):
        return nc.dram_tensor(
            name, shape, dt,
            kind="ExternalOutput" if out else "ExternalInput").ap()

    x_ext = dp("x", [TT, D], F32)
    cmaskT_ext = dp("cmask_t", [TT, S], BF16)   # transposed-block layout
    bct_ext = {c: dp(f"{c}_bct", [TT, 4 * NBC], BF16) for c in ("qk", "v", "know")}
    bcb_ext = {c: dp(f"{c}_bcb", [TT, 8 * D], BF16) for c in ("qk", "v", "know")}
    px_ext = dp("p_x", [TT, 4 * 8], F32)
    ph_ext = dp("p_h", [TT, 4 * 8], F32)
    ob_ext = dp("o_b", [TT, 4 * D], BF16)
    t1_ext = dp("t1", [36, NBC], I16)
    t2_ext = dp("t2", [36, 256], I16)
    out_ext = dp("out", [TT, D], F32, out=True)

    g_scr = nc.dram_tensor("g_scr", [4, TT], I16).ap()
    ccK_in = nc.dram_tensor("ccK_in", [4 * TT * TT], BF16).ap()
    ccK_out = nc.dram_tensor("ccK_out", [n_cores * 4 * TT * TT], BF16,
                             addr_space="Shared").ap()
    ccV_in = nc.dram_tensor("ccV_in", [TT * 8 * 65], BF16).ap()
    ccV_out = nc.dram_tensor("ccV_out", [n_cores * TT * 8 * 65], BF16,
                             addr_space="Shared").ap()

    with TileContext(nc) as tc:
        with (
            tc.tile_pool(name="const", bufs=1) as cpool,
            tc.tile_pool(name="work", bufs=1) as work,
            tc.tile_pool(name="big", bufs=1) as big,
            tc.tile_pool(name="scorep", bufs=2) as scorep,
            tc.tile_pool(name="idxp", bufs=2) as idxp,
            tc.tile_pool(name="att", bufs=2) as att,
            tc.tile_pool(name="ps5", bufs=2, space="PSUM") as ps512,
            tc.tile_pool(name="ps1", bufs=2, space="PSUM") as ps128,
            tc.tile_pool(name="psA", bufs=2, space="PSUM") as psA,
        ):
            k = KB(nc, dict(work=work, big=big, scorep=scorep, idxp=idxp,
                            att=att, ps512=ps512, ps128=ps128, psA=psA))
            k.t1_ext, k.t2_ext = t1_ext, t2_ext
            ident = cpool.tile([TT, TT], F32)
            make_identity(nc, ident)
            k.ident = ident
            ident_b = cpool.tile([TT, TT], BF16)
            nc.vector.tensor_copy(out=ident_b, in_=ident)
            k.ident_b = ident_b
            ones_b = cpool.tile([TT, 1], BF16)
            nc.vector.memset(ones_b, 1.0)

            x_t = cpool.tile([TT, D], F32)
            nc.sync.dma_start(out=x_t, in_=x_ext)
            px_sb = cpool.tile([TT, 4 * 8], F32)
            nc.sync.dma_start(out=px_sb, in_=px_ext)
            ph_sb = cpool.tile([TT, 4 * 8], F32)
            nc.sync.dma_start(out=ph_sb, in_=ph_ext)
            bct, bcb = {}, {}
            for cn in ("qk", "v", "know"):
                bct_full = cpool.tile([TT, 4 * NBC], BF16, tag=f"{cn}bct",
                                      name=f"{cn}bct")
                nc.sync.dma_start(out=bct_full, in_=bct_ext[cn])
                bct[cn] = [bct_full[:, c * NBC:(c + 1) * NBC] for c in range(4)]
                bcb_full = cpool.tile([TT, 8 * D], BF16, tag=f"{cn}bcb",
                                      name=f"{cn}bcb")
                nc.sync.dma_start(out=bcb_full, in_=bcb_ext[cn])
                bcb[cn] = [bcb_full[:, c * D:(c + 1) * D] for c in range(8)]
            ob_sb = cpool.tile([TT, 4 * D], BF16)
            nc.sync.dma_start(out=ob_sb, in_=ob_ext)
            cmaskT = cpool.tile([TT, S], BF16)
            nc.sync.dma_start(out=cmaskT, in_=cmaskT_ext)

            # ---- LN1, transpose, projections
            xn = k.ln(x_t, "ln1")
            xnT, xnTf = k.pe_transpose(xn, 4, "xn", out_dt=BF16, also_f32=True)
            prj = ps128.tile([TT, 8], F32, tag="mm128", name="prj")
            for c in range(4):
                nc.tensor.matmul(prj, xnTf[:, c * TT:(c + 1) * TT],
                                 px_sb[:, c * 8:(c + 1) * 8],
                                 start=(c == 0), stop=(c == 3))
            prj_sb = work.tile([TT, 8], F32, tag="prj_sb", name="prj_sb")
            nc.vector.tensor_copy(out=prj_sb, in_=prj)
            tau3 = prj_sb[:, 4:7]

            g_qk = k.cellgrid(prj_sb[:, 0:2], "cg_qk")
            g_v = k.cellgrid(prj_sb[:, 2:4], "cg_v")
            nc.scalar.dma_start(
                out=g_scr[0].rearrange("(p one) -> p one", one=1), in_=g_qk)
            nc.scalar.dma_start(
                out=g_scr[1].rearrange("(p one) -> p one", one=1), in_=g_v)
            iw_qk = k.wrap_idx(g_scr[0], "qk")
            iw_v = k.wrap_idx(g_scr[1], "v")
            i1_qk, i2_qk = k.gather_tables(iw_qk, "qk")
            i1_v, i2_v = k.gather_tables(iw_v, "v")

            # ---- qk circuit first (K is on the collective critical path)
            s_qk = k.scores(xnT, bct["qk"], "sqk")
            c_qk = k.compact(s_qk, i1_qk, "cqk")
            m0_qk, th_qk = k.topk(c_qk, MAXK_QK, "tqk")
            w_K = k.gate(c_qk, tau3[:, 1:2], m0_qk, th_qk, "gK")
            wt_K = k.expand_transpose(w_K, i2_qk, "wK")
            kt_b = big.tile([TT, 4 * TT], BF16, tag="KT", name="KT")
            for m in range(4):
                ps = ps128.tile([TT, TT], F32, tag="mm128", name=f"ktps{m}")
                for c in range(8):
                    nc.tensor.matmul(ps, bcb["qk"][c][:, m * TT:(m + 1) * TT],
                                     wt_K[:, c * TT:(c + 1) * TT],
                                     start=(c == 0), stop=(c == 7))
                nc.scalar.activation(out=kt_b[:, m * TT:(m + 1) * TT], in_=ps,
                                     func=AF.Copy)
            nc.sync.dma_start(
                out=ccK_in.rearrange("(p n) -> p n", p=TT, n=4 * TT), in_=kt_b)
            if n_cores > 1:
                nc.gpsimd.collective_compute(
                    "AllGather", OP.bypass, ins=[ccK_in], outs=[ccK_out],
                    replica_groups=[list(range(n_cores))])
                kt_gathered = ccK_out
            else:
                kt_gathered = ccK_in

            # ---- remaining gates / V / Q while the K AllGather flies
            w_Q = k.gate(c_qk, tau3[:, 0:1], m0_qk, th_qk, "gQ")
            wt_Q = k.expand_transpose(w_Q, i2_qk, "wQ")
            s_v = k.scores(xnT, bct["v"], "sv")
            c_v = k.compact(s_v, i1_v, "cv")
            m0_v, th_v = k.topk(c_v, MAXK_V, "tv")
            w_V = k.gate(c_v, tau3[:, 2:3], m0_v, th_v, "gV")
            wt_V = k.expand_transpose(w_V, i2_v, "wV")

            def wsum(wt, pl, name):
                ps = ps512.tile([TT, D], F32, tag="mm512", name=f"{name}_ps")
                for c in range(8):
                    nc.tensor.matmul(ps, wt[:, c * TT:(c + 1) * TT], pl[c],
                                     start=(c == 0), stop=(c == 7))
                o = big.tile([TT, D], BF16, tag=f"{name}_o", name=f"{name}_o")
                nc.scalar.activation(out=o, in_=ps, func=AF.Copy)
                return o

            ps_v = ps512.tile([TT, D], F32, tag="mm512", name="V_ps")
            for c in range(8):
                nc.tensor.matmul(ps_v, wt_V[:, c * TT:(c + 1) * TT],
                                 bcb["v"][c], start=(c == 0), stop=(c == 7))
            v_b = big.tile([TT, 8 * 65], BF16, tag="V_o", name="V_o")
            nc.vector.memset(v_b, 1.0)
            for hh in range(NH):
                nc.scalar.activation(out=v_b[:, 65 * hh:65 * hh + 64],
                                     in_=ps_v[:, 64 * hh:64 * hh + 64],
                                     func=AF.Copy)
            nc.sync.dma_start(
                out=ccV_in.rearrange("(p n) -> p n", p=TT, n=8 * 65), in_=v_b)
            if n_cores > 1:
                nc.gpsimd.collective_compute(
                    "AllGather", OP.bypass, ins=[ccV_in], outs=[ccV_out],
                    replica_groups=[list(range(n_cores))])
                v_gathered = ccV_out
            else:
                v_gathered = ccV_in
            q_b = wsum(wt_Q, bcb["qk"], "Q")
            qT = k.pe_transpose(q_b, 4, "q", out_dt=BF16)

            # ---- load gathered K^T / V
            nr = n_cores
            ktv = kt_gathered.rearrange("(r c p t) -> c p r t",
                                        r=nr, c=4, p=TT, t=TT)
            ktf = [big.tile([TT, nr * TT], BF16, tag=f"ktf{c}",
                            name=f"ktf{c}") for c in range(4)]
            for c in range(4):
                eng = nc.sync if (c % 2 == 0) else nc.scalar
                eng.dma_start(out=ktf[c].rearrange("p (r t) -> p r t", t=TT),
                              in_=ktv[c])
            vv = v_gathered.rearrange("(r p n) -> r p n", r=nr, p=TT, n=8 * 65)
            vf = [big.tile([TT, 8 * 65], BF16, tag=f"vf{r}", name=f"vf{r}")
                  for r in range(nr)]
            for r in range(nr):
                eng = nc.sync if (r % 2 == 0) else nc.scalar
                eng.dma_start(out=vf[r], in_=vv[r])

            # ---- attention on transposed scores sT[k, q]
            NK = nr * TT
            aout_b = big.tile([TT, D], BF16, tag="aout", name="aout")
            inv_sqrt = float(1.0 / np.sqrt(DH))
            for h in range(NH):
                ch, po = h // 2, 64 * (h % 2)
                ps_s = psA.tile([TT, NK], F32, tag="att", name=f"attps{h}")
                for r in range(nr):
                    nc.tensor.matmul(
                        ps_s[:, r * TT:(r + 1) * TT],
                        ktf[ch][po:po + 64, r * TT:(r + 1) * TT],
                        qT[po:po + 64, ch * TT:(ch + 1) * TT],
                        start=True, stop=True)
                sb = att.tile([TT, NK], BF16, tag="att_sb", name=f"sb{h}")
                nc.scalar.activation(out=sb, in_=ps_s, func=AF.Copy, bias=0.0,
                                     scale=inv_sqrt)
                sm = att.tile([TT, NK], BF16, tag="att_sm", name=f"sm{h}")
                nc.vector.tensor_tensor(out=sm, in0=sb, in1=cmaskT, op=OP.add)
                at = att.tile([TT, NK], BF16, tag="att_at", name=f"at{h}")
                nc.scalar.activation(out=at, in_=sm, func=AF.Exp)
                ps_a = ps128.tile([TT, 65], F32, tag="mm128", name=f"av{h}")
                for r in range(nr):
                    nc.tensor.matmul(ps_a, at[:, r * TT:(r + 1) * TT],
                                     vf[r][:, 65 * h:65 * h + 65],
                                     start=(r == 0), stop=(r == nr - 1))
                rs = att.tile([TT, 1], F32, tag="att_rs", name=f"rs{h}")
                nc.vector.reciprocal(out=rs, in_=ps_a[:, 64:65])
                nc.vector.tensor_scalar(out=aout_b[:, 64 * h:64 * h + 64],
                                        in0=ps_a[:, 0:64], scalar1=rs,
                                        scalar2=None, op0=OP.mult)

            # ---- expand_O + residual
            aoT = k.pe_transpose(aout_b, 4, "ao", out_dt=BF16)
            ps_o = ps512.tile([TT, D], F32, tag="mm512", name="ps_o")
            for c in range(4):
                nc.tensor.matmul(ps_o, aoT[:, c * TT:(c + 1) * TT],
                                 ob_sb[:, c * D:(c + 1) * D],
                                 start=(c == 0), stop=(c == 3))
            h_t = big.tile([TT, D], F32, tag="h", name="h_t")
            nc.vector.tensor_tensor(out=h_t, in0=x_t, in1=ps_o, op=OP.add)

            # ---- knowledge circuit
            hn = k.ln(h_t, "ln2")
            hnT, hnTf = k.pe_transpose(hn, 4, "hn", out_dt=BF16, also_f32=True)
            prh = ps128.tile([TT, 8], F32, tag="mm128", name="prh")
            for c in range(4):
                nc.tensor.matmul(prh, hnTf[:, c * TT:(c + 1) * TT],
                                 ph_sb[:, c * 8:(c + 1) * 8],
                                 start=(c == 0), stop=(c == 3))
            prh_sb = work.tile([TT, 8], F32, tag="prh_sb", name="prh_sb")
            nc.vector.tensor_copy(out=prh_sb, in_=prh)
            g_k = k.cellgrid(prh_sb[:, 0:2], "cg_k")
            nc.scalar.dma_start(
                out=g_scr[2].rearrange("(p one) -> p one", one=1), in_=g_k)
            iw_k = k.wrap_idx(g_scr[2], "kn")
            i1_k, i2_k = k.gather_tables(iw_k, "kn")
            s_k = k.scores(hnT, bct["know"], "sk")
            c_k = k.compact(s_k, i1_k, "ck")
            m0_k, th_k = k.topk(c_k, MAXK_KNOW, "tk")
            w_KN = k.gate(c_k, prh_sb[:, 2:3], m0_k, th_k, "gKN")
            wt_KN = k.expand_transpose(w_KN, i2_k, "wKN")
            ps_k = ps512.tile([TT, D], F32, tag="mm512", name="ps_k")
            for c in range(8):
                nc.tensor.matmul(ps_k, wt_KN[:, c * TT:(c + 1) * TT],
                                 bcb["know"][c], start=(c == 0), stop=(c == 7))
            out_t = big.tile([TT, D], F32, tag="out", name="out_t")
            nc.vector.tensor_tensor(out=out_t, in0=h_t, in1=ps_k, op=OP.add)
            nc.sync.dma_start(out=out_ext, in_=out_t)

    nc.compile()
    return nc


def _prep_inputs(inputs, n_cores=NCORES):
    T1, T2 = _geometry_tables()
    x = np.ascontiguousarray(inputs["x"].reshape(S, D), dtype=np.float32)
    pools = {}
    for cn, nkey, ikey in (("qk", "qk_neurons", "qk_idx"),
                           ("v", "v_neurons", "v_idx"),
                           ("know", "know_neurons", "know_idx")):
        bc = np.asarray(inputs[nkey], np.float32)[
            np.asarray(inputs[ikey], np.int32).reshape(-1)]
        bctT = bc.T.astype(ml_dtypes.bfloat16)          # [512, 1024]
        pools[f"{cn}_bct"] = np.ascontiguousarray(
            bctT.reshape(4, TT, NBC).transpose(1, 0, 2).reshape(TT, 4 * NBC))
        bcbb = bc.astype(ml_dtypes.bfloat16)            # [1024, 512]
        pools[f"{cn}_bcb"] = np.ascontiguousarray(
            bcbb.reshape(8, TT, D).transpose(1, 0, 2).reshape(TT, 8 * D))
    p_x = np.zeros((D, 8), np.float32)
    p_x[:, 0:2] = inputs["ppqk_k"]
    p_x[:, 2:4] = inputs["ppv_k"]
    p_x[:, 4:7] = inputs["ta_k"]
    p_x = np.ascontiguousarray(
        p_x.reshape(4, TT, 8).transpose(1, 0, 2).reshape(TT, 32))
    p_h = np.zeros((D, 8), np.float32)
    p_h[:, 0:2] = inputs["ppk_k"]
    p_h[:, 2:3] = inputs["tk_k"]
    p_h = np.ascontiguousarray(
        p_h.reshape(4, TT, 8).transpose(1, 0, 2).reshape(TT, 32))
    o_b = np.asarray(inputs["expand_O"], np.float32).astype(ml_dtypes.bfloat16)
    o_b = np.ascontiguousarray(
        o_b.reshape(4, TT, D).transpose(1, 0, 2).reshape(TT, 4 * D))
    in_maps = []
    kpos = np.arange(S)
    for j in range(n_cores):
        qpos = np.arange(TT) + j * TT
        cm = np.where(kpos[None, :] <= qpos[:, None], 0.0, -1e9).astype(
            np.float32)                        # [q, k]
        cmT = np.ascontiguousarray(
            cm.reshape(TT, n_cores, TT).transpose(2, 1, 0).reshape(TT, S)
        ).astype(ml_dtypes.bfloat16)
        in_maps.append({
            "x": x[j * TT:(j + 1) * TT],
            "cmask_t": cmT,
            "p_x": p_x, "p_h": p_h, "o_b": o_b,
            "t1": T1, "t2": T2,
            **pools,
        })
    return in_maps


def kernel(**inputs):
    global LAST_EXEC_NS, LAST_RESULT
    key = ("nc", NCORES)
    if key not in _CACHE:
        _CACHE[key] = build_kernel(NCORES)
    nc = _CACHE[key]
    in_maps = _prep_inputs(inputs, NCORES)
    kw = {}
    if TRACE:
        kw = dict(trace=True, trace_cores=list(range(NCORES)))
    res = run_bass_kernel_spmd(nc, in_maps, core_ids=list(range(NCORES)), **kw)
    LAST_EXEC_NS = res.exec_time_ns
    LAST_RESULT = res
    out = np.concatenate([r["out"] for r in res.results], axis=0)
    return out.reshape(B, S, D).astype(np.float32)


# revision 21
# speedup vs baseline: 1.0895x; 1.0895x over previous
"""Trainium2 Bass kernel for nn_DAWN_35356170781342 (retrieval_knn layer).

kernel(**inputs) takes FULL unsharded inputs (as reference.setup_inputs()
produces) and returns the FULL output [1, 1024, 512] f32.

8-core sequence-parallel design:
  - core j owns tokens [128j, 128j+128)
  - neuron pools re-indexed per cell maps into "bc space" (64 cells x 16
    block = 1024 rows/circuit) on host; replicated per core (transposed-f32
    for score matmuls, bf16 for weighted sums)
  - per-token candidate window = 3x3 cell neighborhood; compaction/expansion
    via GPSIMD local_scatter with per-token index rows dma_gathered from
    small geometry tables
  - exact top-k thresholds via iterative vector.max + match_replace
  - K^T and V (bf16) AllGathered; attention computed on transposed scores
    sT[k, q] so attn needs no transpose; additive mask; exp without max-sub;
    softmax denominator via ones-matmul
"""

import numpy as np
import ml_dtypes

import concourse.bass as bass
import concourse.mybir as mybir
from concourse.bass_types import AP as BassAP
from concourse import bacc
from concourse.tile import TileContext
from concourse.bass_utils import run_bass_kernel_spmd
from concourse.masks import make_identity

F32 = mybir.dt.float32
BF16 = mybir.dt.bfloat16
I16 = mybir.dt.int16
I32 = mybir.dt.int32
AF = mybir.ActivationFunctionType
OP = mybir.AluOpType

B, S, D, NH = 1, 1024, 512, 8
DH = D // NH
CELLS, BLOCK = 8, 16
NBC = CELLS * CELLS * BLOCK          # 1024 bc rows per circuit
NCOMP = 144
MAXK_QK, MAXK_V, MAXK_KNOW = 32, 32, 64
POS_MIN, POS_RANGE = -3.0, 6.0
NCORES = 8
TT = 128

_CACHE = {}
TRACE = False
LAST_EXEC_NS = None
LAST_RESULT = None


def _geometry_tables():
    T1 = np.full((36, NBC), -1, np.int16)
    T2 = np.full((36, 256), 0, np.int16)
    for a in range(1, 7):
        for b in range(1, 7):
            g = (a - 1) * 6 + (b - 1)
            for r in range(3):
                for c in range(3):
                    cell = (a - 1 + r) * 8 + (b - 1 + c)
                    for j in range(16):
                        dense = cell * 16 + j
                        comp = (r * 3 + c) * 16 + j
                        T1[g, dense] = comp
                        T2[g, comp] = dense
    return T1, T2


class KB:
    """Kernel build context."""

    def __init__(self, nc, pools):
        self.nc = nc
        for kk, v in pools.items():
            setattr(self, kk, v)

    def ln(self, x_t, name):
        nc, pool = self.nc, self.work
        ssum = pool.tile([TT, 1], F32, tag=f"{name}_s", name=f"{name}_s")
        nc.vector.reduce_sum(out=ssum, in_=x_t, axis=mybir.AxisListType.X)
        negmean = pool.tile([TT, 1], F32, tag=f"{name}_nm", name=f"{name}_nm")
        nc.vector.tensor_scalar_mul(negmean, ssum, -1.0 / D)
        xc = pool.tile([TT, D], F32, tag="ln_xc", name=f"{name}_xc")
        nc.vector.tensor_scalar(out=xc, in0=x_t, scalar1=negmean, scalar2=None,
                                op0=OP.add)
        sq = pool.tile([TT, D], F32, tag="ln_sq", name=f"{name}_sq")
        sumsq = pool.tile([TT, 1], F32, tag=f"{name}_ss", name=f"{name}_ss")
        nc.scalar.activation(out=sq, in_=xc, func=AF.Square, accum_out=sumsq)
        rstd = pool.tile([TT, 1], F32, tag=f"{name}_rs", name=f"{name}_rs")
        nc.vector.tensor_scalar(out=rstd, in0=sumsq, scalar1=1.0 / D,
                                scalar2=1e-6, op0=OP.mult, op1=OP.add)
        nc.scalar.activation(out=rstd, in_=rstd, func=AF.Sqrt)
        nc.vector.reciprocal(out=rstd, in_=rstd)
        xn = pool.tile([TT, D], F32, tag="ln_xn", name=f"{name}_xn")
        nc.vector.tensor_scalar(out=xn, in0=xc, scalar1=rstd, scalar2=None,
                                op0=OP.mult)
        return xn

    def pe_transpose(self, src, nchunks, name, out_dt=F32, pool=None,
                     also_f32=False):
        nc = self.nc
        pool = pool or self.big
        dst = pool.tile([TT, nchunks * TT], out_dt, tag=f"{name}_T",
                        name=f"{name}_T")
        dst2 = None
        if also_f32:
            dst2 = pool.tile([TT, nchunks * TT], F32, tag=f"{name}_Tf",
                             name=f"{name}_Tf")
        for c in range(nchunks):
            ps = self.ps128.tile([TT, TT], src.dtype, tag="mm128",
                                 name=f"{name}_ps{c}")
            idt = self.ident if src.dtype == F32 else self.ident_b
            nc.tensor.transpose(ps, src[:, c * TT:(c + 1) * TT], idt)
            nc.scalar.activation(out=dst[:, c * TT:(c + 1) * TT], in_=ps,
                                 func=AF.Copy)
            if also_f32:
                nc.vector.tensor_copy(out=dst2[:, c * TT:(c + 1) * TT], in_=ps)
        return (dst, dst2) if also_f32 else dst

    def wrap_idx(self, g_scr_row, name):
        nc = self.nc
        idxw = self.work.tile([128, 8], I16, tag=f"{name}_iw", name=f"{name}_iw")
        src = g_scr_row.rearrange("(s p) -> p s", p=16)
        for kk in range(8):
            nc.gpsimd.dma_start(out=idxw[16 * kk:16 * kk + 16, :], in_=src)
        return idxw

    def gather_tables(self, idxw, name):
        nc = self.nc
        idx1 = self.idxp.tile([128, NBC], I16, tag="i1", name=f"{name}_i1")
        nc.gpsimd.dma_gather(
            out_ap=idx1.rearrange("p (a b) -> p a b", a=1), in_ap=self.t1_ext,
            idxs_ap=idxw, num_idxs=128, num_idxs_reg=128, elem_size=NBC)
        idx2 = self.idxp.tile([128, 256], I16, tag=f"{name}_i2",
                              name=f"{name}_i2")
        nc.gpsimd.dma_gather(
            out_ap=idx2.rearrange("p (a b) -> p a b", a=1), in_ap=self.t2_ext,
            idxs_ap=idxw, num_idxs=128, num_idxs_reg=128, elem_size=256)
        return idx1, idx2

    def scores(self, xnT, bct_tiles, name):
        nc = self.nc
        s = self.scorep.tile([TT, NBC], BF16, tag="scores", name=f"{name}_s")
        for half in range(2):
            ps = self.ps512.tile([TT, 512], F32, tag="mm512",
                                 name=f"{name}_ps{half}")
            for c in range(4):
                nc.tensor.matmul(ps, xnT[:, c * TT:(c + 1) * TT],
                                 bct_tiles[c][:, half * 512:(half + 1) * 512],
                                 start=(c == 0), stop=(c == 3))
            nc.scalar.activation(out=s[:, half * 512:(half + 1) * 512],
                                 in_=ps, func=AF.Copy)
        return s

    def compact(self, s_t, idx1, name):
        nc = self.nc
        compb = self.work.tile([TT, NCOMP], BF16, tag=f"{name}_cb",
                               name=f"{name}_cb")
        nc.gpsimd.local_scatter(
            out_ap=compb, data_ap=s_t, idxs_ap=idx1,
            channels=128, num_elems=NCOMP, num_idxs=NBC)
        comp = self.work.tile([TT, NCOMP], F32, tag=f"{name}_c", name=f"{name}_c")
        nc.vector.tensor_copy(out=comp, in_=compb)
        return comp

    def topk(self, comp, kk, name):
        nc, pool = self.nc, self.work
        work = pool.tile([TT, NCOMP], F32, tag=f"{name}_w", name=f"{name}_w")
        nc.vector.tensor_copy(out=work, in_=comp)
        rounds = kk // 8
        mx = pool.tile([TT, 8 * rounds], F32, tag=f"{name}_m", name=f"{name}_m")
        for r in range(rounds):
            nc.vector.max(out=mx[:, 8 * r:8 * r + 8], in_=work)
            if r < rounds - 1:
                nc.vector.match_replace(out=work,
                                        in_to_replace=mx[:, 8 * r:8 * r + 8],
                                        in_values=work, imm_value=-1e30)
        return mx[:, 0:1], mx[:, 8 * rounds - 1:8 * rounds]

    def gate(self, comp, tau, m0, theta, name):
        nc, pool = self.nc, self.work
        negtau = pool.tile([TT, 1], F32, tag=f"{name}_nt", name=f"{name}_nt")
        nc.vector.tensor_scalar_mul(negtau, tau, -1.0)
        raw = pool.tile([TT, NCOMP], F32, tag=f"{name}_raw", name=f"{name}_raw")
        nc.vector.tensor_scalar(out=raw, in0=comp, scalar1=negtau, scalar2=None,
                                op0=OP.add)
        rmin = pool.tile([TT, NCOMP], F32, tag=f"{name}_rm", name=f"{name}_rm")
        nc.vector.tensor_scalar_min(rmin, raw, 0.0)
        e1 = pool.tile([TT, NCOMP], F32, tag=f"{name}_e1", name=f"{name}_e1")
        nc.scalar.activation(out=e1, in_=rmin, func=AF.Exp)
        nc.vector.tensor_scalar_mul(e1, e1, 1e-8)
        gate = pool.tile([TT, NCOMP], F32, tag=f"{name}_g", name=f"{name}_g")
        nc.vector.tensor_tensor(out=gate, in0=raw, in1=e1, op=OP.max)
        eg1 = pool.tile([TT, NCOMP], F32, tag=f"{name}_eg", name=f"{name}_eg")
        nc.scalar.activation(out=eg1, in_=gate, func=AF.Exp)   # = eg + 1
        keep = pool.tile([TT, NCOMP], F32, tag=f"{name}_k", name=f"{name}_k")
        nc.vector.tensor_scalar(out=keep, in0=comp, scalar1=theta, scalar2=None,
                                op0=OP.is_ge)
        w_u = pool.tile([TT, NCOMP], F32, tag=f"{name}_wu", name=f"{name}_wu")
        nc.vector.tensor_scalar_add(w_u, eg1, -1.0)
        nc.vector.tensor_tensor(out=w_u, in0=w_u, in1=keep, op=OP.mult)
        ssum = pool.tile([TT, 1], F32, tag=f"{name}_ss", name=f"{name}_ss")
        nc.vector.reduce_sum(out=ssum, in_=w_u, axis=mybir.AxisListType.X)
        sc = pool.tile([TT, 4], F32, tag=f"{name}_sc", name=f"{name}_sc")
        rawm = sc[:, 0:1]; rminm = sc[:, 1:2]; egm = sc[:, 2:3]; tnh = sc[:, 3:4]
        nc.vector.tensor_tensor(out=rawm, in0=m0, in1=negtau, op=OP.add)
        nc.vector.tensor_scalar_min(rminm, rawm, 0.0)
        nc.scalar.activation(out=rminm, in_=rminm, func=AF.Exp)
        nc.vector.tensor_scalar_mul(rminm, rminm, 1e-8)
        nc.vector.tensor_tensor(out=egm, in0=rawm, in1=rminm, op=OP.max)
        nc.scalar.activation(out=egm, in_=egm, func=AF.Exp)
        nc.vector.tensor_scalar_add(egm, egm, -1.0)
        nc.scalar.activation(out=tnh, in_=egm, func=AF.Tanh)
        den = pool.tile([TT, 1], F32, tag=f"{name}_dn", name=f"{name}_dn")
        nc.vector.tensor_scalar_add(den, ssum, 1e-8)
        nc.vector.reciprocal(out=den, in_=den)
        scale = pool.tile([TT, 1], F32, tag=f"{name}_scl", name=f"{name}_scl")
        nc.vector.tensor_tensor(out=scale, in0=tnh, in1=den, op=OP.mult)
        wq = pool.tile([TT, NCOMP], F32, tag=f"{name}_wq", name=f"{name}_wq")
        nc.vector.tensor_tensor(out=wq, in0=w_u, in1=comp, op=OP.mult)
        wb = pool.tile([TT, NCOMP], BF16, tag=f"{name}_wb", name=f"{name}_wb")
        nc.vector.tensor_scalar(out=wb, in0=wq, scalar1=scale, scalar2=None,
                                op0=OP.mult)
        return wb

    def expand_transpose(self, wc_b, idx2, name):
        nc = self.nc
        wd = self.idxp.tile([TT, NBC], BF16, tag="wd", name=f"{name}_wd")
        nc.gpsimd.local_scatter(out_ap=wd, data_ap=wc_b,
                                idxs_ap=idx2[:, 0:NCOMP], channels=128,
                                num_elems=NBC, num_idxs=NCOMP)
        return self.pe_transpose(wd, 8, f"{name}w", out_dt=BF16)

    def cellgrid(self, pos2, name):
        nc, pool = self.nc, self.work
        q = pool.tile([TT, 2], F32, tag=f"{name}_q", name=f"{name}_q")
        nc.scalar.activation(out=q, in_=pos2, func=AF.Copy, bias=3.5,
                             scale=float(CELLS / POS_RANGE))
        ci = pool.tile([TT, 2], I32, tag=f"{name}_ci", name=f"{name}_ci")
        nc.vector.tensor_copy(out=ci, in_=q)
        nc.vector.tensor_scalar_max(ci, ci, 1)
        nc.vector.tensor_scalar_min(ci, ci, 6)
        g = pool.tile([TT, 1], I32, tag=f"{name}_gi", name=f"{name}_gi")
        cx6 = pool.tile([TT, 1], I32, tag=f"{name}_c6", name=f"{name}_c6")
        nc.vector.tensor_scalar(out=cx6, in0=ci[:, 0:1], scalar1=6, scalar2=-7,
                                op0=OP.mult, op1=OP.add)
        nc.vector.tensor_tensor(out=g, in0=cx6, in1=ci[:, 1:2], op=OP.add)
        g16 = pool.tile([TT, 1], I16, tag=f"{name}_g16", name=f"{name}_g16")
        nc.vector.tensor_copy(out=g16, in_=g)
        return g16


def build_kernel(n_cores=NCORES, debug=False):
    nc = bacc.Bacc("TRN2", target_bir_lowering=False, debug=False,
                   num_devices=n_cores)

    def dp(name, shape, dt, out=False):
        return nc.dram_tensor(
            name, shape, dt,
            kind="ExternalOutput" if out else "ExternalInput").ap()

    x_ext = dp("x", [TT, D], F32)
    cmaskT_ext = dp("cmask_t", [TT, S], BF16)   # transposed-block layout
    bct_ext = {c: dp(f"{c}_bct", [TT, 4 * NBC], BF16) for c in ("qk", "v", "know")}
    bcb_ext = {c: dp(f"{c}_bcb", [TT, 8 * D], BF16) for c in ("qk", "v", "know")}
    px_ext = dp("p_x", [TT, 4 * 8], F32)
    ph_ext = dp("p_h", [TT, 4 * 8], F32)
    ob_ext = dp("o_b", [TT, 4 * D], BF16)
    t1_ext = dp("t1", [36, NBC], I16)
    t2_ext = dp("t2", [36, 256], I16)
    out_ext = dp("out", [TT, D], F32, out=True)

    g_scr = nc.dram_tensor("g_scr", [4, TT], I16).ap()
    ccK_in = nc.dram_tensor("ccK_in", [4 * TT * TT], BF16).ap()
    ccK_out = nc.dram_tensor("ccK_out", [n_cores * 4 * TT * TT], BF16,
                             addr_space="Shared").ap()
    ccV_in = nc.dram_tensor("ccV_in", [TT * 8 * 65], BF16).ap()
    ccV_out = nc.dram_tensor("ccV_out", [n_cores * TT * 8 * 65], BF16,
                             addr_space="Shared").ap()

    with TileContext(nc) as tc:
        with (
            tc.tile_pool(name="const", bufs=1) as cpool,
            tc.tile_pool(name="work", bufs=1) as work,
            tc.tile_pool(name="big", bufs=1) as big,
            tc.tile_pool(name="scorep", bufs=2) as scorep,
            tc.tile_pool(name="idxp", bufs=2) as idxp,
            tc.tile_pool(name="att", bufs=2) as att,
            tc.tile_pool(name="ps5", bufs=2, space="PSUM") as ps512,
            tc.tile_pool(name="ps1", bufs=2, space="PSUM") as ps128,
            tc.tile_pool(name="psA", bufs=2, space="PSUM") as psA,
        ):
            k = KB(nc, dict(work=work, big=big, scorep=scorep, idxp=idxp,
                            att=att, ps512=ps512, ps128=ps128, psA=psA))
            k.t1_ext, k.t2_ext = t1_ext, t2_ext
            ident = cpool.tile([TT, TT], F32)
            make_identity(nc, ident)
            k.ident = ident
            ident_b = cpool.tile([TT, TT], BF16)
            nc.vector.tensor_copy(out=ident_b, in_=ident)
            k.ident_b = ident_b
            ones_b = cpool.tile([TT, 1], BF16)
            nc.vector.memset(ones_b, 1.0)

            x_t = cpool.tile([TT, D], F32)
            nc.sync.dma_start(out=x_t, in_=x_ext)
            px_sb = cpool.tile([TT, 4 * 8], F32)
            nc.sync.dma_start(out=px_sb, in_=px_ext)
            ph_sb = cpool.tile([TT, 4 * 8], F32)
            nc.sync.dma_start(out=ph_sb, in_=ph_ext)
            bct, bcb = {}, {}

            def load_pool(cn, which):
                if which == "bct":
                    t = cpool.tile([TT, 4 * NBC], BF16, tag=f"{cn}bct",
                                   name=f"{cn}bct")
                    nc.sync.dma_start(out=t, in_=bct_ext[cn])
                    bct[cn] = [t[:, c * NBC:(c + 1) * NBC] for c in range(4)]
                else:
                    t = cpool.tile([TT, 8 * D], BF16, tag=f"{cn}bcb",
                                   name=f"{cn}bcb")
                    nc.sync.dma_start(out=t, in_=bcb_ext[cn])
                    bcb[cn] = [t[:, c * D:(c + 1) * D] for c in range(8)]

            load_pool("qk", "bct")
            load_pool("qk", "bcb")

            # ---- LN1, transpose, projections
            xn = k.ln(x_t, "ln1")
            xnT, xnTf = k.pe_transpose(xn, 4, "xn", out_dt=BF16, also_f32=True)
            prj = ps128.tile([TT, 8], F32, tag="mm128", name="prj")
            for c in range(4):
                nc.tensor.matmul(prj, xnTf[:, c * TT:(c + 1) * TT],
                                 px_sb[:, c * 8:(c + 1) * 8],
                                 start=(c == 0), stop=(c == 3))
            prj_sb = work.tile([TT, 8], F32, tag="prj_sb", name="prj_sb")
            nc.vector.tensor_copy(out=prj_sb, in_=prj)
            tau3 = prj_sb[:, 4:7]

            g_qk = k.cellgrid(prj_sb[:, 0:2], "cg_qk")
            g_v = k.cellgrid(prj_sb[:, 2:4], "cg_v")
            nc.gpsimd.dma_start(
                out=g_scr[0].rearrange("(p one) -> p one", one=1), in_=g_qk)
            nc.gpsimd.dma_start(
                out=g_scr[1].rearrange("(p one) -> p one", one=1), in_=g_v)
            iw_qk = k.wrap_idx(g_scr[0], "qk")
            iw_v = k.wrap_idx(g_scr[1], "v")
            i1_qk, i2_qk = k.gather_tables(iw_qk, "qk")
            i1_v, i2_v = k.gather_tables(iw_v, "v")

            # ---- qk circuit first (K is on the collective critical path)
            s_qk = k.scores(xnT, bct["qk"], "sqk")
            load_pool("v", "bct")
            load_pool("v", "bcb")
            c_qk = k.compact(s_qk, i1_qk, "cqk")
            m0_qk, th_qk = k.topk(c_qk, MAXK_QK, "tqk")
            w_K = k.gate(c_qk, tau3[:, 1:2], m0_qk, th_qk, "gK")
            wt_K = k.expand_transpose(w_K, i2_qk, "wK")
            kt_b = big.tile([TT, 4 * TT], BF16, tag="KT", name="KT")
            for m in range(4):
                ps = ps128.tile([TT, TT], F32, tag="mm128", name=f"ktps{m}")
                for c in range(8):
                    nc.tensor.matmul(ps, bcb["qk"][c][:, m * TT:(m + 1) * TT],
                                     wt_K[:, c * TT:(c + 1) * TT],
                                     start=(c == 0), stop=(c == 7))
                nc.scalar.activation(out=kt_b[:, m * TT:(m + 1) * TT], in_=ps,
                                     func=AF.Copy)
            nc.sync.dma_start(
                out=ccK_in.rearrange("(p n) -> p n", p=TT, n=4 * TT), in_=kt_b)
            if n_cores > 1:
                nc.gpsimd.collective_compute(
                    "AllGather", OP.bypass, ins=[ccK_in], outs=[ccK_out],
                    replica_groups=[list(range(n_cores))])
                kt_gathered = ccK_out
            else:
                kt_gathered = ccK_in

            # ---- remaining gates / V / Q while the K AllGather flies
            w_Q = k.gate(c_qk, tau3[:, 0:1], m0_qk, th_qk, "gQ")
            wt_Q = k.expand_transpose(w_Q, i2_qk, "wQ")
            s_v = k.scores(xnT, bct["v"], "sv")
            c_v = k.compact(s_v, i1_v, "cv")
            m0_v, th_v = k.topk(c_v, MAXK_V, "tv")
            w_V = k.gate(c_v, tau3[:, 2:3], m0_v, th_v, "gV")
            wt_V = k.expand_transpose(w_V, i2_v, "wV")

            def wsum(wt, pl, name):
                ps = ps512.tile([TT, D], F32, tag="mm512", name=f"{name}_ps")
                for c in range(8):
                    nc.tensor.matmul(ps, wt[:, c * TT:(c + 1) * TT], pl[c],
                                     start=(c == 0), stop=(c == 7))
                o = big.tile([TT, D], BF16, tag=f"{name}_o", name=f"{name}_o")
                nc.scalar.activation(out=o, in_=ps, func=AF.Copy)
                return o

            load_pool("know", "bct")
            load_pool("know", "bcb")
            ob_sb = cpool.tile([TT, 4 * D], BF16)
            nc.sync.dma_start(out=ob_sb, in_=ob_ext)
            cmaskT = cpool.tile([TT, S], BF16)
            nc.sync.dma_start(out=cmaskT, in_=cmaskT_ext)
            ps_v = ps512.tile([TT, D], F32, tag="mm512", name="V_ps")
            for c in range(8):
                nc.tensor.matmul(ps_v, wt_V[:, c * TT:(c + 1) * TT],
                                 bcb["v"][c], start=(c == 0), stop=(c == 7))
            v_b = big.tile([TT, 8 * 65], BF16, tag="V_o", name="V_o")
            nc.vector.memset(v_b, 1.0)
            for hh in range(NH):
                nc.scalar.activation(out=v_b[:, 65 * hh:65 * hh + 64],
                                     in_=ps_v[:, 64 * hh:64 * hh + 64],
                                     func=AF.Copy)
            nc.sync.dma_start(
                out=ccV_in.rearrange("(p n) -> p n", p=TT, n=8 * 65), in_=v_b)
            if n_cores > 1:
                nc.gpsimd.collective_compute(
                    "AllGather", OP.bypass, ins=[ccV_in], outs=[ccV_out],
                    replica_groups=[list(range(n_cores))])
                v_gathered = ccV_out
            else:
                v_gathered = ccV_in
            q_b = wsum(wt_Q, bcb["qk"], "Q")
            qT = k.pe_transpose(q_b, 4, "q", out_dt=BF16)

            # ---- load gathered K^T / V
            nr = n_cores
            ktv = kt_gathered.rearrange("(r c p t) -> c p r t",
                                        r=nr, c=4, p=TT, t=TT)
            ktf = [big.tile([TT, nr * TT], BF16, tag=f"ktf{c}",
                            name=f"ktf{c}") for c in range(4)]
            for c in range(4):
                eng = nc.sync if (c % 2 == 0) else nc.scalar
                eng.dma_start(out=ktf[c].rearrange("p (r t) -> p r t", t=TT),
                              in_=ktv[c])
            vv = v_gathered.rearrange("(r p n) -> r p n", r=nr, p=TT, n=8 * 65)
            vf = [big.tile([TT, 8 * 65], BF16, tag=f"vf{r}", name=f"vf{r}")
                  for r in range(nr)]
            for r in range(nr):
                eng = nc.sync if (r % 2 == 0) else nc.scalar
                eng.dma_start(out=vf[r], in_=vv[r])

            # ---- attention on transposed scores sT[k, q]
            NK = nr * TT
            aout_b = big.tile([TT, D], BF16, tag="aout", name="aout")
            inv_sqrt = float(1.0 / np.sqrt(DH))
            for h in range(NH):
                ch, po = h // 2, 64 * (h % 2)
                ps_s = psA.tile([TT, NK], F32, tag="att", name=f"attps{h}")
                for r in range(nr):
                    nc.tensor.matmul(
                        ps_s[:, r * TT:(r + 1) * TT],
                        ktf[ch][po:po + 64, r * TT:(r + 1) * TT],
                        qT[po:po + 64, ch * TT:(ch + 1) * TT],
                        start=True, stop=True)
                sb = att.tile([TT, NK], BF16, tag="att_sb", name=f"sb{h}")
                nc.scalar.activation(out=sb, in_=ps_s, func=AF.Copy, bias=0.0,
                                     scale=inv_sqrt)
                sm = att.tile([TT, NK], BF16, tag="att_sm", name=f"sm{h}")
                nc.vector.tensor_tensor(out=sm, in0=sb, in1=cmaskT, op=OP.add)
                at = att.tile([TT, NK], BF16, tag="att_at", name=f"at{h}")
                nc.scalar.activation(out=at, in_=sm, func=AF.Exp)
                ps_a = ps128.tile([TT, 65], F32, tag="mm128", name=f"av{h}")
                for r in range(nr):
                    nc.tensor.matmul(ps_a, at[:, r * TT:(r + 1) * TT],
                                     vf[r][:, 65 * h:65 * h + 65],
                                     start=(r == 0), stop=(r == nr - 1))
                rs = att.tile([TT, 1], F32, tag="att_rs", name=f"rs{h}")
                nc.vector.reciprocal(out=rs, in_=ps_a[:, 64:65])
                nc.vector.tensor_scalar(out=aout_b[:, 64 * h:64 * h + 64],
                                        in0=ps_a[:, 0:64], scalar1=rs,
                                        scalar2=None, op0=OP.mult)

            # ---- expand_O + residual
            aoT = k.pe_transpose(aout_b, 4, "ao", out_dt=BF16)
            ps_o = ps512.tile([TT, D], F32, tag="mm512", name="ps_o")
            for c in range(4):
                nc.tensor.matmul(ps_o, aoT[:, c * TT:(c + 1) * TT],
                                 ob_sb[:, c * D:(c + 1) * D],
                                 start=(c == 0), stop=(c == 3))
            h_t = big.tile([TT, D], F32, tag="h", name="h_t")
            nc.vector.tensor_tensor(out=h_t, in0=x_t, in1=ps_o, op=OP.add)

            # ---- knowledge circuit
            hn = k.ln(h_t, "ln2")
            hnT, hnTf = k.pe_transpose(hn, 4, "hn", out_dt=BF16, also_f32=True)
            prh = ps128.tile([TT, 8], F32, tag="mm128", name="prh")
            for c in range(4):
                nc.tensor.matmul(prh, hnTf[:, c * TT:(c + 1) * TT],
                                 ph_sb[:, c * 8:(c + 1) * 8],
                                 start=(c == 0), stop=(c == 3))
            prh_sb = work.tile([TT, 8], F32, tag="prh_sb", name="prh_sb")
            nc.vector.tensor_copy(out=prh_sb, in_=prh)
            g_k = k.cellgrid(prh_sb[:, 0:2], "cg_k")
            nc.gpsimd.dma_start(
                out=g_scr[2].rearrange("(p one) -> p one", one=1), in_=g_k)
            iw_k = k.wrap_idx(g_scr[2], "kn")
            i1_k, i2_k = k.gather_tables(iw_k, "kn")
            s_k = k.scores(hnT, bct["know"], "sk")
            c_k = k.compact(s_k, i1_k, "ck")
            m0_k, th_k = k.topk(c_k, MAXK_KNOW, "tk")
            w_KN = k.gate(c_k, prh_sb[:, 2:3], m0_k, th_k, "gKN")
            wt_KN = k.expand_transpose(w_KN, i2_k, "wKN")
            ps_k = ps512.tile([TT, D], F32, tag="mm512", name="ps_k")
            for c in range(8):
                nc.tensor.matmul(ps_k, wt_KN[:, c * TT:(c + 1) * TT],
                                 bcb["know"][c], start=(c == 0), stop=(c == 7))
            out_t = big.tile([TT, D], F32, tag="out", name="out_t")
            nc.vector.tensor_tensor(out=out_t, in0=h_t, in1=ps_k, op=OP.add)
            nc.sync.dma_start(out=out_ext, in_=out_t)

    nc.compile()
    return nc


def _prep_inputs(inputs, n_cores=NCORES):
    T1, T2 = _geometry_tables()
    x = np.ascontiguousarray(inputs["x"].reshape(S, D), dtype=np.float32)
    pools = {}
    for cn, nkey, ikey in (("qk", "qk_neurons", "qk_idx"),
                           ("v", "v_neurons", "v_idx"),
                           ("know", "know_neurons", "know_idx")):
        bc = np.asarray(inputs[nkey], np.float32)[
            np.asarray(inputs[ikey], np.int32).reshape(-1)]
        bctT = bc.T.astype(ml_dtypes.bfloat16)          # [512, 1024]
        pools[f"{cn}_bct"] = np.ascontiguousarray(
            bctT.reshape(4, TT, NBC).transpose(1, 0, 2).reshape(TT, 4 * NBC))
        bcbb = bc.astype(ml_dtypes.bfloat16)            # [1024, 512]
        pools[f"{cn}_bcb"] = np.ascontiguousarray(
            bcbb.reshape(8, TT, D).transpose(1, 0, 2).reshape(TT, 8 * D))
    p_x = np.zeros((D, 8), np.float32)
    p_x[:, 0:2] = inputs["ppqk_k"]
    p_x[:, 2:4] = inputs["ppv_k"]
    p_x[:, 4:7] = inputs["ta_k"]
    p_x = np.ascontiguousarray(
        p_x.reshape(4, TT, 8).transpose(1, 0, 2).reshape(TT, 32))
    p_h = np.zeros((D, 8), np.float32)
    p_h[:, 0:2] = inputs["ppk_k"]
    p_h[:, 2:3] = inputs["tk_k"]
    p_h = np.ascontiguousarray(
        p_h.reshape(4, TT, 8).transpose(1, 0, 2).reshape(TT, 32))
    o_b = np.asarray(inputs["expand_O"], np.float32).astype(ml_dtypes.bfloat16)
    o_b = np.ascontiguousarray(
        o_b.reshape(4, TT, D).transpose(1, 0, 2).reshape(TT, 4 * D))
    in_maps = []
    kpos = np.arange(S)
    for j in range(n_cores):
        qpos = np.arange(TT) + j * TT
        cm = np.where(kpos[None, :] <= qpos[:, None], 0.0, -1e9).astype(
            np.float32)                        # [q, k]
        cmT = np.ascontiguousarray(
            cm.reshape(TT, n_cores, TT).transpose(2, 1, 0).reshape(TT, S)
        ).astype(ml_dtypes.bfloat16)
        in_maps.append({
            "x": x[j * TT:(j + 1) * TT],
            "cmask_t": cmT,
            "p_x": p_x, "p_h": p_h, "o_b": o_b,
            "t1": T1, "t2": T2,
            **pools,
        })
    return in_maps


def kernel(**inputs):
    global LAST_EXEC_NS, LAST_RESULT
    key = ("nc", NCORES)
    if key not in _CACHE:
        _CACHE[key] = build_kernel(NCORES)
    nc = _CACHE[key]
    in_maps = _prep_inputs(inputs, NCORES)
    kw = {}
    if TRACE:
        kw = dict(trace=True, trace_cores=list(range(NCORES)))
    res = run_bass_kernel_spmd(nc, in_maps, core_ids=list(range(NCORES)), **kw)
    LAST_EXEC_NS = res.exec_time_ns
    LAST_RESULT = res
    out = np.concatenate([r["out"] for r in res.results], axis=0)
    return out.reshape(B, S, D).astype(np.float32)


# revision 23
# speedup vs baseline: 1.7891x; 1.6421x over previous
"""Trainium2 Bass kernel for nn_DAWN_35356170781342 (retrieval_knn layer).

kernel(**inputs) takes FULL unsharded inputs (as reference.setup_inputs()
produces) and returns the FULL output [1, 1024, 512] f32.

8-core sequence-parallel design:
  - core j owns tokens [128j, 128j+128)
  - neuron pools re-indexed per cell maps into "bc space" (64 cells x 16
    block = 1024 rows/circuit) on host; replicated per core (transposed-f32
    for score matmuls, bf16 for weighted sums)
  - per-token candidate window = 3x3 cell neighborhood; compaction/expansion
    via GPSIMD local_scatter with per-token index rows dma_gathered from
    small geometry tables
  - exact top-k thresholds via iterative vector.max + match_replace
  - K^T and V (bf16) AllGathered; attention computed on transposed scores
    sT[k, q] so attn needs no transpose; additive mask; exp without max-sub;
    softmax denominator via ones-matmul
"""

import numpy as np
import ml_dtypes

import concourse.bass as bass
import concourse.mybir as mybir
from concourse.bass_types import AP as BassAP
from concourse import bacc
from concourse.tile import TileContext
from concourse.bass_utils import run_bass_kernel_spmd
from concourse.masks import make_identity

F32 = mybir.dt.float32
BF16 = mybir.dt.bfloat16
I16 = mybir.dt.int16
I32 = mybir.dt.int32
AF = mybir.ActivationFunctionType
OP = mybir.AluOpType

B, S, D, NH = 1, 1024, 512, 8
DH = D // NH
CELLS, BLOCK = 8, 16
NBC = CELLS * CELLS * BLOCK          # 1024 bc rows per circuit
NCOMP = 144
MAXK_QK, MAXK_V, MAXK_KNOW = 32, 32, 64
POS_MIN, POS_RANGE = -3.0, 6.0
NCORES = 8
TT = 128

_CACHE = {}
TRACE = False
LAST_EXEC_NS = None
LAST_RESULT = None


def _geometry_tables():
    T1 = np.full((36, NBC), -1, np.int16)
    T2 = np.full((36, 256), 0, np.int16)
    for a in range(1, 7):
        for b in range(1, 7):
            g = (a - 1) * 6 + (b - 1)
            for r in range(3):
                for c in range(3):
                    cell = (a - 1 + r) * 8 + (b - 1 + c)
                    for j in range(16):
                        dense = cell * 16 + j
                        comp = (r * 3 + c) * 16 + j
                        T1[g, dense] = comp
                        T2[g, comp] = dense
    return T1, T2


class KB:
    """Kernel build context."""

    def __init__(self, nc, pools):
        self.nc = nc
        for kk, v in pools.items():
            setattr(self, kk, v)

    def ln(self, x_t, name):
        nc, pool = self.nc, self.work
        ssum = pool.tile([TT, 1], F32, tag=f"{name}_s", name=f"{name}_s")
        nc.vector.reduce_sum(out=ssum, in_=x_t, axis=mybir.AxisListType.X)
        negmean = pool.tile([TT, 1], F32, tag=f"{name}_nm", name=f"{name}_nm")
        nc.vector.tensor_scalar_mul(negmean, ssum, -1.0 / D)
        xc = pool.tile([TT, D], F32, tag="ln_xc", name=f"{name}_xc")
        nc.vector.tensor_scalar(out=xc, in0=x_t, scalar1=negmean, scalar2=None,
                                op0=OP.add)
        sq = pool.tile([TT, D], F32, tag="ln_sq", name=f"{name}_sq")
        sumsq = pool.tile([TT, 1], F32, tag=f"{name}_ss", name=f"{name}_ss")
        nc.scalar.activation(out=sq, in_=xc, func=AF.Square, accum_out=sumsq)
        rstd = pool.tile([TT, 1], F32, tag=f"{name}_rs", name=f"{name}_rs")
        nc.vector.tensor_scalar(out=rstd, in0=sumsq, scalar1=1.0 / D,
                                scalar2=1e-6, op0=OP.mult, op1=OP.add)
        nc.scalar.activation(out=rstd, in_=rstd, func=AF.Sqrt)
        nc.vector.reciprocal(out=rstd, in_=rstd)
        xn = pool.tile([TT, D], F32, tag="ln_xn", name=f"{name}_xn")
        nc.vector.tensor_scalar(out=xn, in0=xc, scalar1=rstd, scalar2=None,
                                op0=OP.mult)
        return xn

    def pe_transpose(self, src, nchunks, name, out_dt=F32, pool=None,
                     also_f32=False):
        nc = self.nc
        pool = pool or self.big
        dst = pool.tile([TT, nchunks * TT], out_dt, tag=f"{name}_T",
                        name=f"{name}_T")
        dst2 = None
        if also_f32:
            dst2 = pool.tile([TT, nchunks * TT], F32, tag=f"{name}_Tf",
                             name=f"{name}_Tf")
        for c in range(nchunks):
            ps = self.ps128.tile([TT, TT], src.dtype, tag="mm128",
                                 name=f"{name}_ps{c}")
            idt = self.ident if src.dtype == F32 else self.ident_b
            nc.tensor.transpose(ps, src[:, c * TT:(c + 1) * TT], idt)
            nc.scalar.activation(out=dst[:, c * TT:(c + 1) * TT], in_=ps,
                                 func=AF.Copy)
            if also_f32:
                nc.vector.tensor_copy(out=dst2[:, c * TT:(c + 1) * TT], in_=ps)
        return (dst, dst2) if also_f32 else dst

    def idx_tables(self, g_f32, name):
        """i1[t,:] = T1[g_t,:], i2[t,:] = T2[g_t,:] via one-hot matmuls."""
        nc = self.nc
        oh = self.work.tile([TT, TT], BF16, tag="oh", name=f"{name}_oh")
        nc.vector.tensor_scalar(out=oh, in0=self.rgrid, scalar1=g_f32,
                                scalar2=None, op0=OP.is_equal)
        ohT = self.pe_transpose(oh, 1, f"{name}oh", out_dt=BF16,
                                pool=self.work, also_f32=True)
        ohT_b, ohT_f = ohT
        i1 = self.idxp.tile([128, NBC], I16, tag="i1", name=f"{name}_i1")
        for half in range(2):
            ps = self.ps512.tile([TT, 512], F32, tag="mm512",
                                 name=f"{name}_i1ps{half}")
            nc.tensor.matmul(ps, ohT_b, self.t1_sb[:, half * 512:(half + 1) * 512],
                             start=True, stop=True)
            nc.vector.tensor_copy(out=i1[:, half * 512:(half + 1) * 512], in_=ps)
        i2 = self.idxp.tile([128, 256], I16, tag=f"{name}_i2", name=f"{name}_i2")
        ps2 = self.ps128.tile([TT, 256], F32, tag="mm128", name=f"{name}_i2ps")
        nc.tensor.matmul(ps2, ohT_f, self.t2_sb, start=True, stop=True)
        nc.vector.tensor_copy(out=i2, in_=ps2)
        return i1, i2

    def scores(self, xnT, bct_tiles, name):
        nc = self.nc
        s = self.scorep.tile([TT, NBC], BF16, tag="scores", name=f"{name}_s")
        for half in range(2):
            ps = self.ps512.tile([TT, 512], F32, tag="mm512",
                                 name=f"{name}_ps{half}")
            for c in range(4):
                nc.tensor.matmul(ps, xnT[:, c * TT:(c + 1) * TT],
                                 bct_tiles[c][:, half * 512:(half + 1) * 512],
                                 start=(c == 0), stop=(c == 3))
            nc.scalar.activation(out=s[:, half * 512:(half + 1) * 512],
                                 in_=ps, func=AF.Copy)
        return s

    def compact(self, s_t, idx1, name):
        nc = self.nc
        compb = self.work.tile([TT, NCOMP], BF16, tag=f"{name}_cb",
                               name=f"{name}_cb")
        nc.gpsimd.local_scatter(
            out_ap=compb, data_ap=s_t, idxs_ap=idx1,
            channels=128, num_elems=NCOMP, num_idxs=NBC)
        comp = self.work.tile([TT, NCOMP], F32, tag=f"{name}_c", name=f"{name}_c")
        nc.vector.tensor_copy(out=comp, in_=compb)
        return comp

    def topk(self, comp, kk, name):
        nc, pool = self.nc, self.work
        work = pool.tile([TT, NCOMP], F32, tag=f"{name}_w", name=f"{name}_w")
        nc.vector.tensor_copy(out=work, in_=comp)
        rounds = kk // 8
        mx = pool.tile([TT, 8 * rounds], F32, tag=f"{name}_m", name=f"{name}_m")
        for r in range(rounds):
            nc.vector.max(out=mx[:, 8 * r:8 * r + 8], in_=work)
            if r < rounds - 1:
                nc.vector.match_replace(out=work,
                                        in_to_replace=mx[:, 8 * r:8 * r + 8],
                                        in_values=work, imm_value=-1e30)
        return mx[:, 0:1], mx[:, 8 * rounds - 1:8 * rounds]

    def gate(self, comp, tau, m0, theta, name):
        nc, pool = self.nc, self.work
        negtau = pool.tile([TT, 1], F32, tag=f"{name}_nt", name=f"{name}_nt")
        nc.vector.tensor_scalar_mul(negtau, tau, -1.0)
        raw = pool.tile([TT, NCOMP], F32, tag=f"{name}_raw", name=f"{name}_raw")
        nc.vector.tensor_scalar(out=raw, in0=comp, scalar1=negtau, scalar2=None,
                                op0=OP.add)
        rmin = pool.tile([TT, NCOMP], F32, tag=f"{name}_rm", name=f"{name}_rm")
        nc.vector.tensor_scalar_min(rmin, raw, 0.0)
        e1 = pool.tile([TT, NCOMP], F32, tag=f"{name}_e1", name=f"{name}_e1")
        nc.scalar.activation(out=e1, in_=rmin, func=AF.Exp)
        nc.vector.tensor_scalar_mul(e1, e1, 1e-8)
        gate = pool.tile([TT, NCOMP], F32, tag=f"{name}_g", name=f"{name}_g")
        nc.vector.tensor_tensor(out=gate, in0=raw, in1=e1, op=OP.max)
        eg1 = pool.tile([TT, NCOMP], F32, tag=f"{name}_eg", name=f"{name}_eg")
        nc.scalar.activation(out=eg1, in_=gate, func=AF.Exp)   # = eg + 1
        keep = pool.tile([TT, NCOMP], F32, tag=f"{name}_k", name=f"{name}_k")
        nc.vector.tensor_scalar(out=keep, in0=comp, scalar1=theta, scalar2=None,
                                op0=OP.is_ge)
        w_u = pool.tile([TT, NCOMP], F32, tag=f"{name}_wu", name=f"{name}_wu")
        nc.vector.tensor_scalar_add(w_u, eg1, -1.0)
        nc.vector.tensor_tensor(out=w_u, in0=w_u, in1=keep, op=OP.mult)
        ssum = pool.tile([TT, 1], F32, tag=f"{name}_ss", name=f"{name}_ss")
        nc.vector.reduce_sum(out=ssum, in_=w_u, axis=mybir.AxisListType.X)
        sc = pool.tile([TT, 4], F32, tag=f"{name}_sc", name=f"{name}_sc")
        rawm = sc[:, 0:1]; rminm = sc[:, 1:2]; egm = sc[:, 2:3]; tnh = sc[:, 3:4]
        nc.vector.tensor_tensor(out=rawm, in0=m0, in1=negtau, op=OP.add)
        nc.vector.tensor_scalar_min(rminm, rawm, 0.0)
        nc.scalar.activation(out=rminm, in_=rminm, func=AF.Exp)
        nc.vector.tensor_scalar_mul(rminm, rminm, 1e-8)
        nc.vector.tensor_tensor(out=egm, in0=rawm, in1=rminm, op=OP.max)
        nc.scalar.activation(out=egm, in_=egm, func=AF.Exp)
        nc.vector.tensor_scalar_add(egm, egm, -1.0)
        nc.scalar.activation(out=tnh, in_=egm, func=AF.Tanh)
        den = pool.tile([TT, 1], F32, tag=f"{name}_dn", name=f"{name}_dn")
        nc.vector.tensor_scalar_add(den, ssum, 1e-8)
        nc.vector.reciprocal(out=den, in_=den)
        scale = pool.tile([TT, 1], F32, tag=f"{name}_scl", name=f"{name}_scl")
        nc.vector.tensor_tensor(out=scale, in0=tnh, in1=den, op=OP.mult)
        wq = pool.tile([TT, NCOMP], F32, tag=f"{name}_wq", name=f"{name}_wq")
        nc.vector.tensor_tensor(out=wq, in0=w_u, in1=comp, op=OP.mult)
        wb = pool.tile([TT, NCOMP], BF16, tag=f"{name}_wb", name=f"{name}_wb")
        nc.vector.tensor_scalar(out=wb, in0=wq, scalar1=scale, scalar2=None,
                                op0=OP.mult)
        return wb

    def expand_transpose(self, wc_b, idx2, name):
        nc = self.nc
        wd = self.idxp.tile([TT, NBC], BF16, tag="wd", name=f"{name}_wd")
        nc.gpsimd.local_scatter(out_ap=wd, data_ap=wc_b,
                                idxs_ap=idx2[:, 0:NCOMP], channels=128,
                                num_elems=NBC, num_idxs=NCOMP)
        return self.pe_transpose(wd, 8, f"{name}w", out_dt=BF16)

    def cellgrid(self, pos2, name):
        nc, pool = self.nc, self.work
        q = pool.tile([TT, 2], F32, tag=f"{name}_q", name=f"{name}_q")
        nc.scalar.activation(out=q, in_=pos2, func=AF.Copy, bias=3.5,
                             scale=float(CELLS / POS_RANGE))
        ci = pool.tile([TT, 2], I32, tag=f"{name}_ci", name=f"{name}_ci")
        nc.vector.tensor_copy(out=ci, in_=q)
        nc.vector.tensor_scalar_max(ci, ci, 1)
        nc.vector.tensor_scalar_min(ci, ci, 6)
        g = pool.tile([TT, 1], I32, tag=f"{name}_gi", name=f"{name}_gi")
        cx6 = pool.tile([TT, 1], I32, tag=f"{name}_c6", name=f"{name}_c6")
        nc.vector.tensor_scalar(out=cx6, in0=ci[:, 0:1], scalar1=6, scalar2=-7,
                                op0=OP.mult, op1=OP.add)
        nc.vector.tensor_tensor(out=g, in0=cx6, in1=ci[:, 1:2], op=OP.add)
        gf = pool.tile([TT, 1], F32, tag=f"{name}_gf", name=f"{name}_gf")
        nc.vector.tensor_copy(out=gf, in_=g)
        return gf


def build_kernel(n_cores=NCORES, debug=False):
    nc = bacc.Bacc("TRN2", target_bir_lowering=False, debug=False,
                   num_devices=n_cores)

    def dp(name, shape, dt, out=False):
        return nc.dram_tensor(
            name, shape, dt,
            kind="ExternalOutput" if out else "ExternalInput").ap()

    x_ext = dp("x", [TT, D], F32)
    cmaskT_ext = dp("cmask_t", [TT, S], BF16)   # transposed-block layout
    bct_ext = {c: dp(f"{c}_bct", [TT, 4 * NBC], BF16) for c in ("qk", "v", "know")}
    bcb_ext = {c: dp(f"{c}_bcb", [TT, 8 * D], BF16) for c in ("qk", "v", "know")}
    px_ext = dp("p_x", [TT, 4 * 8], F32)
    ph_ext = dp("p_h", [TT, 4 * 8], F32)
    ob_ext = dp("o_b", [TT, 4 * D], BF16)
    t1_ext = dp("t1", [TT, NBC], BF16)
    t2_ext = dp("t2", [TT, 256], F32)
    out_ext = dp("out", [TT, D], F32, out=True)

    ccK_in = nc.dram_tensor("ccK_in", [4 * TT * TT], BF16).ap()
    ccK_out = nc.dram_tensor("ccK_out", [n_cores * 4 * TT * TT], BF16,
                             addr_space="Shared").ap()
    ccV_in = nc.dram_tensor("ccV_in", [TT * 8 * 65], BF16).ap()
    ccV_out = nc.dram_tensor("ccV_out", [n_cores * TT * 8 * 65], BF16,
                             addr_space="Shared").ap()

    with TileContext(nc) as tc:
        with (
            tc.tile_pool(name="const", bufs=1) as cpool,
            tc.tile_pool(name="work", bufs=1) as work,
            tc.tile_pool(name="big", bufs=1) as big,
            tc.tile_pool(name="scorep", bufs=2) as scorep,
            tc.tile_pool(name="idxp", bufs=2) as idxp,
            tc.tile_pool(name="att", bufs=2) as att,
            tc.tile_pool(name="ps5", bufs=2, space="PSUM") as ps512,
            tc.tile_pool(name="ps1", bufs=2, space="PSUM") as ps128,
            tc.tile_pool(name="psA", bufs=2, space="PSUM") as psA,
        ):
            k = KB(nc, dict(work=work, big=big, scorep=scorep, idxp=idxp,
                            att=att, ps512=ps512, ps128=ps128, psA=psA))
            k.t1_ext, k.t2_ext = t1_ext, t2_ext
            ident = cpool.tile([TT, TT], F32)
            make_identity(nc, ident)
            k.ident = ident
            ident_b = cpool.tile([TT, TT], BF16)
            nc.vector.tensor_copy(out=ident_b, in_=ident)
            k.ident_b = ident_b
            ones_b = cpool.tile([TT, 1], BF16)
            nc.vector.memset(ones_b, 1.0)
            rgrid_i = cpool.tile([TT, TT], I32)
            nc.gpsimd.iota(out=rgrid_i, pattern=[[1, TT]], base=0,
                           channel_multiplier=0)
            rgrid = cpool.tile([TT, TT], F32)
            nc.vector.tensor_copy(out=rgrid, in_=rgrid_i)
            k.rgrid = rgrid
            t1_sb = cpool.tile([TT, NBC], BF16)
            nc.sync.dma_start(out=t1_sb, in_=t1_ext)
            k.t1_sb = t1_sb
            t2_sb = cpool.tile([TT, 256], F32)
            nc.sync.dma_start(out=t2_sb, in_=t2_ext)
            k.t2_sb = t2_sb

            x_t = cpool.tile([TT, D], F32)
            nc.sync.dma_start(out=x_t, in_=x_ext)
            px_sb = cpool.tile([TT, 4 * 8], F32)
            nc.sync.dma_start(out=px_sb, in_=px_ext)
            ph_sb = cpool.tile([TT, 4 * 8], F32)
            nc.sync.dma_start(out=ph_sb, in_=ph_ext)
            bct, bcb = {}, {}

            def load_pool(cn, which):
                if which == "bct":
                    t = cpool.tile([TT, 4 * NBC], BF16, tag=f"{cn}bct",
                                   name=f"{cn}bct")
                    nc.sync.dma_start(out=t, in_=bct_ext[cn])
                    bct[cn] = [t[:, c * NBC:(c + 1) * NBC] for c in range(4)]
                else:
                    t = cpool.tile([TT, 8 * D], BF16, tag=f"{cn}bcb",
                                   name=f"{cn}bcb")
                    nc.sync.dma_start(out=t, in_=bcb_ext[cn])
                    bcb[cn] = [t[:, c * D:(c + 1) * D] for c in range(8)]

            load_pool("qk", "bct")
            load_pool("qk", "bcb")

            # ---- LN1, transpose, projections
            xn = k.ln(x_t, "ln1")
            xnT, xnTf = k.pe_transpose(xn, 4, "xn", out_dt=BF16, also_f32=True)
            prj = ps128.tile([TT, 8], F32, tag="mm128", name="prj")
            for c in range(4):
                nc.tensor.matmul(prj, xnTf[:, c * TT:(c + 1) * TT],
                                 px_sb[:, c * 8:(c + 1) * 8],
                                 start=(c == 0), stop=(c == 3))
            prj_sb = work.tile([TT, 8], F32, tag="prj_sb", name="prj_sb")
            nc.vector.tensor_copy(out=prj_sb, in_=prj)
            tau3 = prj_sb[:, 4:7]

            g_qk = k.cellgrid(prj_sb[:, 0:2], "cg_qk")
            g_v = k.cellgrid(prj_sb[:, 2:4], "cg_v")
            i1_qk, i2_qk = k.idx_tables(g_qk, "qk")
            i1_v, i2_v = k.idx_tables(g_v, "v")

            # ---- qk circuit first (K is on the collective critical path)
            s_qk = k.scores(xnT, bct["qk"], "sqk")
            load_pool("v", "bct")
            load_pool("v", "bcb")
            c_qk = k.compact(s_qk, i1_qk, "cqk")
            m0_qk, th_qk = k.topk(c_qk, MAXK_QK, "tqk")
            w_K = k.gate(c_qk, tau3[:, 1:2], m0_qk, th_qk, "gK")
            wt_K = k.expand_transpose(w_K, i2_qk, "wK")
            kt_b = big.tile([TT, 4 * TT], BF16, tag="KT", name="KT")
            for m in range(4):
                ps = ps128.tile([TT, TT], F32, tag="mm128", name=f"ktps{m}")
                for c in range(8):
                    nc.tensor.matmul(ps, bcb["qk"][c][:, m * TT:(m + 1) * TT],
                                     wt_K[:, c * TT:(c + 1) * TT],
                                     start=(c == 0), stop=(c == 7))
                nc.scalar.activation(out=kt_b[:, m * TT:(m + 1) * TT], in_=ps,
                                     func=AF.Copy)
            nc.sync.dma_start(
                out=ccK_in.rearrange("(p n) -> p n", p=TT, n=4 * TT), in_=kt_b)
            if n_cores > 1:
                nc.gpsimd.collective_compute(
                    "AllGather", OP.bypass, ins=[ccK_in], outs=[ccK_out],
                    replica_groups=[list(range(n_cores))])
                kt_gathered = ccK_out
            else:
                kt_gathered = ccK_in

            # ---- remaining gates / V / Q while the K AllGather flies
            w_Q = k.gate(c_qk, tau3[:, 0:1], m0_qk, th_qk, "gQ")
            wt_Q = k.expand_transpose(w_Q, i2_qk, "wQ")
            s_v = k.scores(xnT, bct["v"], "sv")
            c_v = k.compact(s_v, i1_v, "cv")
            m0_v, th_v = k.topk(c_v, MAXK_V, "tv")
            w_V = k.gate(c_v, tau3[:, 2:3], m0_v, th_v, "gV")
            wt_V = k.expand_transpose(w_V, i2_v, "wV")

            def wsum(wt, pl, name):
                ps = ps512.tile([TT, D], F32, tag="mm512", name=f"{name}_ps")
                for c in range(8):
                    nc.tensor.matmul(ps, wt[:, c * TT:(c + 1) * TT], pl[c],
                                     start=(c == 0), stop=(c == 7))
                o = big.tile([TT, D], BF16, tag=f"{name}_o", name=f"{name}_o")
                nc.scalar.activation(out=o, in_=ps, func=AF.Copy)
                return o

            load_pool("know", "bct")
            load_pool("know", "bcb")
            ob_sb = cpool.tile([TT, 4 * D], BF16)
            nc.sync.dma_start(out=ob_sb, in_=ob_ext)
            cmaskT = cpool.tile([TT, S], BF16)
            nc.sync.dma_start(out=cmaskT, in_=cmaskT_ext)
            ps_v = ps512.tile([TT, D], F32, tag="mm512", name="V_ps")
            for c in range(8):
                nc.tensor.matmul(ps_v, wt_V[:, c * TT:(c + 1) * TT],
                                 bcb["v"][c], start=(c == 0), stop=(c == 7))
            v_b = big.tile([TT, 8 * 65], BF16, tag="V_o", name="V_o")
            nc.vector.memset(v_b, 1.0)
            for hh in range(NH):
                nc.scalar.activation(out=v_b[:, 65 * hh:65 * hh + 64],
                                     in_=ps_v[:, 64 * hh:64 * hh + 64],
                                     func=AF.Copy)
            nc.sync.dma_start(
                out=ccV_in.rearrange("(p n) -> p n", p=TT, n=8 * 65), in_=v_b)
            if n_cores > 1:
                nc.gpsimd.collective_compute(
                    "AllGather", OP.bypass, ins=[ccV_in], outs=[ccV_out],
                    replica_groups=[list(range(n_cores))])
                v_gathered = ccV_out
            else:
                v_gathered = ccV_in
            q_b = wsum(wt_Q, bcb["qk"], "Q")
            qT = k.pe_transpose(q_b, 4, "q", out_dt=BF16)

            # ---- load gathered K^T / V
            nr = n_cores
            ktv = kt_gathered.rearrange("(r c p t) -> c p r t",
                                        r=nr, c=4, p=TT, t=TT)
            ktf = [big.tile([TT, nr * TT], BF16, tag=f"ktf{c}",
                            name=f"ktf{c}") for c in range(4)]
            for c in range(4):
                eng = nc.sync if (c % 2 == 0) else nc.scalar
                eng.dma_start(out=ktf[c].rearrange("p (r t) -> p r t", t=TT),
                              in_=ktv[c])
            vv = v_gathered.rearrange("(r p n) -> r p n", r=nr, p=TT, n=8 * 65)
            vf = [big.tile([TT, 8 * 65], BF16, tag=f"vf{r}", name=f"vf{r}")
                  for r in range(nr)]
            for r in range(nr):
                eng = nc.sync if (r % 2 == 0) else nc.scalar
                eng.dma_start(out=vf[r], in_=vv[r])

            # ---- attention on transposed scores sT[k, q]
            NK = nr * TT
            aout_b = big.tile([TT, D], BF16, tag="aout", name="aout")
            inv_sqrt = float(1.0 / np.sqrt(DH))
            for h in range(NH):
                ch, po = h // 2, 64 * (h % 2)
                ps_s = psA.tile([TT, NK], F32, tag="att", name=f"attps{h}")
                for r in range(nr):
                    nc.tensor.matmul(
                        ps_s[:, r * TT:(r + 1) * TT],
                        ktf[ch][po:po + 64, r * TT:(r + 1) * TT],
                        qT[po:po + 64, ch * TT:(ch + 1) * TT],
                        start=True, stop=True)
                sb = att.tile([TT, NK], BF16, tag="att_sb", name=f"sb{h}")
                nc.scalar.activation(out=sb, in_=ps_s, func=AF.Copy, bias=0.0,
                                     scale=inv_sqrt)
                sm = att.tile([TT, NK], BF16, tag="att_sm", name=f"sm{h}")
                nc.vector.tensor_tensor(out=sm, in0=sb, in1=cmaskT, op=OP.add)
                at = att.tile([TT, NK], BF16, tag="att_at", name=f"at{h}")
                nc.scalar.activation(out=at, in_=sm, func=AF.Exp)
                ps_a = ps128.tile([TT, 65], F32, tag="mm128", name=f"av{h}")
                for r in range(nr):
                    nc.tensor.matmul(ps_a, at[:, r * TT:(r + 1) * TT],
                                     vf[r][:, 65 * h:65 * h + 65],
                                     start=(r == 0), stop=(r == nr - 1))
                rs = att.tile([TT, 1], F32, tag="att_rs", name=f"rs{h}")
                nc.vector.reciprocal(out=rs, in_=ps_a[:, 64:65])
                nc.vector.tensor_scalar(out=aout_b[:, 64 * h:64 * h + 64],
                                        in0=ps_a[:, 0:64], scalar1=rs,
                                        scalar2=None, op0=OP.mult)

            # ---- expand_O + residual
            aoT = k.pe_transpose(aout_b, 4, "ao", out_dt=BF16)
            ps_o = ps512.tile([TT, D], F32, tag="mm512", name="ps_o")
            for c in range(4):
                nc.tensor.matmul(ps_o, aoT[:, c * TT:(c + 1) * TT],
                                 ob_sb[:, c * D:(c + 1) * D],
                                 start=(c == 0), stop=(c == 3))
            h_t = big.tile([TT, D], F32, tag="h", name="h_t")
            nc.vector.tensor_tensor(out=h_t, in0=x_t, in1=ps_o, op=OP.add)

            # ---- knowledge circuit
            hn = k.ln(h_t, "ln2")
            hnT, hnTf = k.pe_transpose(hn, 4, "hn", out_dt=BF16, also_f32=True)
            prh = ps128.tile([TT, 8], F32, tag="mm128", name="prh")
            for c in range(4):
                nc.tensor.matmul(prh, hnTf[:, c * TT:(c + 1) * TT],
                                 ph_sb[:, c * 8:(c + 1) * 8],
                                 start=(c == 0), stop=(c == 3))
            prh_sb = work.tile([TT, 8], F32, tag="prh_sb", name="prh_sb")
            nc.vector.tensor_copy(out=prh_sb, in_=prh)
            g_k = k.cellgrid(prh_sb[:, 0:2], "cg_k")
            i1_k, i2_k = k.idx_tables(g_k, "kn")
            s_k = k.scores(hnT, bct["know"], "sk")
            c_k = k.compact(s_k, i1_k, "ck")
            m0_k, th_k = k.topk(c_k, MAXK_KNOW, "tk")
            w_KN = k.gate(c_k, prh_sb[:, 2:3], m0_k, th_k, "gKN")
            wt_KN = k.expand_transpose(w_KN, i2_k, "wKN")
            ps_k = ps512.tile([TT, D], F32, tag="mm512", name="ps_k")
            for c in range(8):
                nc.tensor.matmul(ps_k, wt_KN[:, c * TT:(c + 1) * TT],
                                 bcb["know"][c], start=(c == 0), stop=(c == 7))
            out_t = big.tile([TT, D], F32, tag="out", name="out_t")
            nc.vector.tensor_tensor(out=out_t, in0=h_t, in1=ps_k, op=OP.add)
            nc.sync.dma_start(out=out_ext, in_=out_t)

    nc.compile()
    return nc


def _prep_inputs(inputs, n_cores=NCORES):
    T1, T2 = _geometry_tables()
    T1b = np.zeros((TT, NBC), ml_dtypes.bfloat16)
    T1b[:36] = T1.astype(np.float32).astype(ml_dtypes.bfloat16)
    T2f = np.zeros((TT, 256), np.float32)
    T2f[:36] = T2.astype(np.float32)
    x = np.ascontiguousarray(inputs["x"].reshape(S, D), dtype=np.float32)
    pools = {}
    for cn, nkey, ikey in (("qk", "qk_neurons", "qk_idx"),
                           ("v", "v_neurons", "v_idx"),
                           ("know", "know_neurons", "know_idx")):
        bc = np.asarray(inputs[nkey], np.float32)[
            np.asarray(inputs[ikey], np.int32).reshape(-1)]
        bctT = bc.T.astype(ml_dtypes.bfloat16)          # [512, 1024]
        pools[f"{cn}_bct"] = np.ascontiguousarray(
            bctT.reshape(4, TT, NBC).transpose(1, 0, 2).reshape(TT, 4 * NBC))
        bcbb = bc.astype(ml_dtypes.bfloat16)            # [1024, 512]
        pools[f"{cn}_bcb"] = np.ascontiguousarray(
            bcbb.reshape(8, TT, D).transpose(1, 0, 2).reshape(TT, 8 * D))
    p_x = np.zeros((D, 8), np.float32)
    p_x[:, 0:2] = inputs["ppqk_k"]
    p_x[:, 2:4] = inputs["ppv_k"]
    p_x[:, 4:7] = inputs["ta_k"]
    p_x = np.ascontiguousarray(
        p_x.reshape(4, TT, 8).transpose(1, 0, 2).reshape(TT, 32))
    p_h = np.zeros((D, 8), np.float32)
    p_h[:, 0:2] = inputs["ppk_k"]
    p_h[:, 2:3] = inputs["tk_k"]
    p_h = np.ascontiguousarray(
        p_h.reshape(4, TT, 8).transpose(1, 0, 2).reshape(TT, 32))
    o_b = np.asarray(inputs["expand_O"], np.float32).astype(ml_dtypes.bfloat16)
    o_b = np.ascontiguousarray(
        o_b.reshape(4, TT, D).transpose(1, 0, 2).reshape(TT, 4 * D))
    in_maps = []
    kpos = np.arange(S)
    for j in range(n_cores):
        qpos = np.arange(TT) + j * TT
        cm = np.where(kpos[None, :] <= qpos[:, None], 0.0, -1e9).astype(
            np.float32)                        # [q, k]
        cmT = np.ascontiguousarray(
            cm.reshape(TT, n_cores, TT).transpose(2, 1, 0).reshape(TT, S)
        ).astype(ml_dtypes.bfloat16)
        in_maps.append({
            "x": x[j * TT:(j + 1) * TT],
            "cmask_t": cmT,
            "p_x": p_x, "p_h": p_h, "o_b": o_b,
            "t1": T1b, "t2": T2f,
            **pools,
        })
    return in_maps


def kernel(**inputs):
    global LAST_EXEC_NS, LAST_RESULT
    key = ("nc", NCORES)
    if key not in _CACHE:
        _CACHE[key] = build_kernel(NCORES)
    nc = _CACHE[key]
    in_maps = _prep_inputs(inputs, NCORES)
    kw = {}
    if TRACE:
        kw = dict(trace=True, trace_cores=list(range(NCORES)))
    res = run_bass_kernel_spmd(nc, in_maps, core_ids=list(range(NCORES)), **kw)
    LAST_EXEC_NS = res.exec_time_ns
    LAST_RESULT = res
    out = np.concatenate([r["out"] for r in res.results], axis=0)
    return out.reshape(B, S, D).astype(np.float32)
